# revision 5
# baseline (speedup 1.0000x reference)
"""Multi-head attention (B=2, T=2048, D=1024, H=16) on 8 Trainium2 NeuronCores.

Sharding: tensor-parallel over heads — core c owns global heads {2c, 2c+1} for
both batch elements (Wq/Wk/Wv column-split, Wo row-split, relpos_bias split
along H).  Each core computes a partial [B, D, T] output-projection product;
the host sums the 8 partials and transposes back to [B, T, D].  SPMD: one
program, per-core weight/relpos slices in the input maps; no collectives.

Device-side layout ("transposed flash attention"): scores are computed as
S^T[k, q] so the exp'd scores are already in the right layout (k on
partitions) to be the moving operand of the P@V matmul — the attention
matrix is never transposed on device.

v2 design notes (changes vs the 269us baseline):
  - relpos bias is applied MULTIPLICATIVELY after exp: p = exp(s)*exp(r),
    with exp(relpos^T) precomputed on host in fp16 (causal-masked entries
    are exactly 0).  The multiply runs on DVE in 4x mode (all-SBUF, all
    fp16) — one fused [128, w_tot] multiply per k-chunk — replacing the
    fp8 identity-matmul adds that cost ~29us of PE time in the baseline.
  - weights are pre-transposed on host into contiguous per-partition
    layouts (2KB rows) so the initial weight DMA takes ~1us instead of
    ~16us of 256B-descriptor gather; first-needed-first DMA ordering gets
    the first projection matmul issued within a few us.
  - the two heads' output projections are merged: at_sb is one [128, T]
    tile per batch (h0 rows 0:64, h1 rows 64:128) and Wo is one [128, D]
    stationary, halving oproj matmul count (128 -> 64).
  - normalization drops the hi/lo fp16 split (fp16 reciprocal broadcast is
    ~5e-4 rel err, far inside the 2e-2 budget): denom-row copy + fast
    reciprocal + fp16 cast on DVE, one ones-outer-product matmul on PE,
    numerator multiply on DVE reading PSUM directly.
  - fp16 matmuls everywhere; softmax max-subtraction skipped (scores are
    ~N(0,1), exp is safe); denominator comes free as an extra row of the
    P@V matmul from an all-ones column in vaug; 1/sqrt(dk) folded into Wq.
  - engine assignment: PE matmuls; Scalar exp only; DVE copies/norm/
    relpos-mul; GpSimd batch-1 prefetch DMA issues + output-cast copies;
    Sync relpos/output/input DMA issues.
  - the whole program is one software-pipelined stream: batch-1
    projections, per-q-group normalizations, and output-projection pieces
    are interleaved into the attention k-loops so the PE never idles.
"""

import sys

for p in ("/opt/trn_rl_repo", "/root/.axon_site/_ro/trn_rl_repo"):
    if p not in sys.path:
        sys.path.insert(0, p)

import numpy as np

import concourse.bacc as bacc
import concourse.mybir as mybir
import concourse.tile as tile
from concourse.bass_utils import run_bass_kernel_spmd

B, T, D, H = 2, 2048, 1024, 16
DK = D // H          # 64
NCORES = 8
HPC = H // NCORES    # heads per core = 2
QG = 512             # q-group width
NQG = T // QG        # 4
NKC = T // 128       # 16 k-chunks
NDC = D // 128       # 8 d-chunks
NEG = np.float32(-1e30)

F32 = mybir.dt.float32
FP16 = mybir.dt.float16

_CACHE = {}


def _build_program():
    nc = bacc.Bacc("TRN2", target_bir_lowering=False, debug=False,
                   enable_asserts=True)

    d_qT = nc.dram_tensor("qT", [B, D, T], FP16, kind="ExternalInput").ap()
    d_kT = nc.dram_tensor("kT", [B, D, T], FP16, kind="ExternalInput").ap()
    d_vT = nc.dram_tensor("vT", [B, D, T], FP16, kind="ExternalInput").ap()
    d_rp = nc.dram_tensor("exprpT", [HPC, T, T], FP16, kind="ExternalInput").ap()
    d_kp = nc.dram_tensor("kpadT", [128, B, NKC], F32, kind="ExternalInput").ap()
    d_wq = nc.dram_tensor("wqT", [128, D], FP16, kind="ExternalInput").ap()
    d_wk = nc.dram_tensor("wkT", [128, D], FP16, kind="ExternalInput").ap()
    d_wv = nc.dram_tensor("wvT", [128, D], FP16, kind="ExternalInput").ap()
    d_wo = nc.dram_tensor("woT", [128, D], FP16, kind="ExternalInput").ap()
    d_out = nc.dram_tensor("outT", [B, D, T], FP16, kind="ExternalOutput").ap()

    with tile.TileContext(nc) as tc:
        with (
            tc.tile_pool(name="persist", bufs=1) as persist,
            tc.tile_pool(name="stream", bufs=6) as stream,
            tc.tile_pool(name="rp", bufs=5) as rppool,
            tc.tile_pool(name="ee", bufs=3) as epool,
            tc.tile_pool(name="pp", bufs=3) as ppool,
            tc.tile_pool(name="oc", bufs=3) as ocpool,
            tc.tile_pool(name="nrm", bufs=2) as nrm,
            tc.tile_pool(name="ps", bufs=4, space="PSUM") as ps,
            tc.tile_pool(name="opsum", bufs=4, space="PSUM") as ops,
        ):
            # ---- weights + constants, first-needed-first on two rings ----
            wq = persist.tile([128, NDC, 128], FP16, tag="wq", name="wq")
            nc.scalar.dma_start(out=wq[:], in_=d_wq.rearrange(
                "p (a m) -> p a m", m=128))

            def load_x(dten, b, dk, eng, tag="xin", bufs=10):
                t = stream.tile([128, T], FP16, tag=tag, bufs=bufs,
                                name=f"x{tag}{b}{dk}")
                eng.dma_start(out=t[:],
                              in_=dten[b, dk * 128:(dk + 1) * 128, :])
                return t

            engs = (nc.scalar, nc.sync)
            xq0 = {dk: load_x(d_qT, 0, dk, engs[dk % 2]) for dk in range(NDC)}

            wv = persist.tile([128, NDC, 128], FP16, tag="wv", name="wv")
            nc.scalar.dma_start(out=wv[:], in_=d_wv.rearrange(
                "p (a m) -> p a m", m=128))
            vch0 = {dk: load_x(d_vT, 0, dk, engs[dk % 2], "xinv", 8)
                    for dk in range(NDC)}

            wk = persist.tile([128, NDC, 128], FP16, tag="wk", name="wk")
            nc.sync.dma_start(out=wk[:], in_=d_wk.rearrange(
                "p (a m) -> p a m", m=128))
            xk0 = {dk: load_x(d_kT, 0, dk, engs[dk % 2]) for dk in range(NDC)}

            w_sb = {"q": wq, "k": wk, "v": wv}

            kpad = persist.tile([128, B, NKC], F32, tag="kpad", name="kpad")
            nc.sync.dma_start(out=kpad[:], in_=d_kp[:])
            wo_sb = persist.tile([128, D], FP16, tag="wo", name="wo")
            nc.sync.dma_start(out=wo_sb[:], in_=d_wo[:])

            ones = persist.tile([128, DK], F32, tag="ones", name="ones")
            nc.vector.memset(ones[:], 1.0)
            ones16 = persist.tile([128, DK], FP16, tag="ones16", name="ones16")
            nc.vector.memset(ones16[:], 1.0)

            qt_sb, kt_sb, vaug = {}, {}, {}
            for b in range(B):
                qt_sb[b] = persist.tile([128, T], FP16, tag=f"qt{b}",
                                        name=f"qt{b}")
                kt_sb[b] = persist.tile([128, T], FP16, tag=f"kt{b}",
                                        name=f"kt{b}")
                va = persist.tile([128, HPC, NKC * 80], FP16, tag=f"va{b}",
                                  name=f"va{b}")
                va_c = va[:].rearrange("p h (c u) -> p h c u", u=80)
                for h in range(HPC):
                    nc.vector.tensor_copy(va_c[:, h, :, 64], ones[:, 0:NKC])
                vaug[b] = va

            # ---- projection helpers (weight-stationary, dk outer) ----
            def proj_qk(nm, b, xts, dst):
                accs = [ps.tile([128, QG], F32, tag="ps", name="ps")
                        for _ in range(NQG)]
                for dk in range(NDC):
                    for cc in range(NQG):
                        nc.tensor.matmul(
                            accs[cc][:], w_sb[nm][:, dk, :],
                            xts[dk][:, cc * QG:(cc + 1) * QG],
                            start=(dk == 0), stop=(dk == NDC - 1))
                for cc in range(NQG):
                    nc.vector.tensor_copy(
                        dst[b][:, cc * QG:(cc + 1) * QG], accs[cc][:])

            def proj_v(b, vts):
                for tb in range(NKC):
                    ts_ = slice(tb * 128, (tb + 1) * 128)
                    acc = ps.tile([128, 128], F32, tag="ps", name="psv")
                    for dk in range(NDC):
                        nc.tensor.matmul(
                            acc[:], vts[dk][:, ts_], w_sb["v"][:, dk, :],
                            start=(dk == 0), stop=(dk == NDC - 1))
                    # both heads' 64-wide slices in one strided copy
                    va_c = vaug[b][:].rearrange("p h (c u) -> p h c u", u=80)
                    nc.vector.tensor_copy(
                        va_c[:, :, tb, 0:DK],
                        acc[:].rearrange("p (h u) -> p h u", h=HPC))

            # ---- phase 1: batch 0 projections ----
            proj_qk("q", 0, xq0, qt_sb)
            proj_v(0, vch0)
            proj_qk("k", 0, xk0, kt_sb)

            # ---- phase 2: attention; batch-1 projections and all output
            # projections are interleaved into the instruction stream ----
            at_sb = {}
            for b in range(B):
                at_sb[b] = persist.tile([128, T], FP16, tag=f"at{b}",
                                        name=f"at{b}")

            def norm_dve(u, qg):
                # full-tile copy + reciprocal: recip needs an SBUF source at
                # partition base 0 (single-partition slices at base 64
                # silently return zeros), and the numerator multiply wants
                # the SBUF copy anyway.  fp16 cast feeds the broadcast
                # matmul — 5e-4 rel err, far inside budget, so no hi/lo.
                o = nrm.tile([DK + 1, QG], F32, tag="oc2", name="oc2")
                nc.vector.tensor_copy(o[:], pend_ops[u][qg][:])
                rc = nrm.tile([DK + 1, QG], F32, tag="rc", name="rc")
                nc.vector.reciprocal_approx_fast(out=rc[:], in_=o[:])
                rch = nrm.tile([DK + 1, QG], FP16, tag="rch", name="rch")
                nc.vector.tensor_copy(rch[:], rc[:])
                pend_dve[(u, qg)] = (o, rch)

            def norm_pe(u, qg):
                b, h = u
                o, rch = pend_dve.pop((u, qg))
                rb = ops.tile([DK, QG], F32, tag="ops", name="rb")
                nc.tensor.matmul(
                    rb[:], ones16[DK:DK + 1, :], rch[DK:DK + 1, :],
                    start=True, stop=True)
                nc.vector.tensor_mul(
                    at_sb[b][h * DK:(h + 1) * DK, qg * QG:(qg + 1) * QG],
                    o[0:DK, :], rb[:])

            oproj_n = [0]

            def oproj_piece(b, db, qg):
                ds_ = slice(db * 128, (db + 1) * 128)
                qs = slice(qg * QG, (qg + 1) * QG)
                pp = ps.tile([128, QG], F32, tag="ps", name="pp")
                nc.tensor.matmul(pp[:], wo_sb[:, ds_], at_sb[b][:, qs],
                                 start=True, stop=True)
                oc = ocpool.tile([128, QG], FP16, tag="oc", name="oc")
                # PSUM->SBUF cast: gpsimd can't touch PSUM, so alternate
                # between DVE and the Scalar engine's Copy activation
                # (same act table as Exp — no table reload)
                if oproj_n[0] % 2 == 0:
                    nc.vector.tensor_copy(oc[:], pp[:])
                else:
                    nc.scalar.activation(oc[:], pp[:],
                                         mybir.ActivationFunctionType.Copy)
                oproj_n[0] += 1
                nc.sync.dma_start(out=d_out[b, ds_, qs], in_=oc[:])

            pend_ops, pend_dve = {}, {}
            pending = []          # (unit, qg) whose norm_pe is still owed
            oproj_q = []          # (b, db, qg) out-proj pieces ready to emit
            units = [(b, h) for b in range(B) for h in range(HPC)]
            last_u = units[-1]
            xq1 = vch1 = None
            for ui, u in enumerate(units):
                b, h = u
                hs = slice(h * DK, (h + 1) * DK)
                if ui == 0:
                    # prefetch batch-1 q and v rows.  These issues can wait
                    # a long time (in engine terms) on pool-slot semaphores,
                    # so they MUST NOT sit in the Scalar queue where they
                    # would stall the exp instructions behind them; the
                    # GpSimd (SWDGE) queue absorbs the waits (its only later
                    # work — output casts — starts after the slots free).
                    xq1 = {dk: load_x(d_qT, 1, dk, nc.gpsimd)
                           for dk in range(NDC)}
                    vch1 = {dk: load_x(d_vT, 1, dk, nc.gpsimd, "xinv", 8)
                            for dk in range(NDC)}
                if ui == 1:
                    # batch-1 projections: data already resident
                    proj_qk("q", 1, xq1, qt_sb)
                    xk1 = {dk: load_x(d_kT, 1, dk, nc.gpsimd)
                           for dk in range(NDC)}
                    proj_v(1, vch1)
                    proj_qk("k", 1, xk1, kt_sb)
                pend_ops[u] = [ops.tile([DK + 1, QG], F32, tag="ops",
                                        name="ops") for _ in range(NQG)]
                o_ps = pend_ops[u]
                for ck in range(NKC):
                    if ck == 4 and pending:
                        norm_pe(*pending.pop(0))
                    if ck >= 5 and (ck - 5) % 4 == 0:
                        qgn = (ck - 5) // 4
                        norm_pe(u, qgn)
                        if u == last_u and qgn < NQG - 1:
                            oproj_q.extend((1, db, qgn) for db in range(NDC))
                    if ck >= 4 and oproj_q:
                        oproj_piece(*oproj_q.pop(0))
                        if len(oproj_q) > 24 - (NKC - ck):
                            oproj_piece(*oproj_q.pop(0))
                    qg0 = ck // 4          # first valid q-group
                    off = qg0 * QG         # start col of rp tile
                    co0 = ck * 128 - off   # first valid col within tile
                    wt = T - off - co0     # total valid width this k-chunk
                    rp_t = rppool.tile([128, T], FP16, tag="rp", name="rp")
                    nc.sync.dma_start(
                        out=rp_t[:, 0:T - off],
                        in_=d_rp[h, ck * 128:(ck + 1) * 128, off:T])
                    s_ts, cos, ws = [], [], []
                    for qg in range(qg0, NQG):
                        co = max(0, ck * 128 - qg * QG)
                        w = QG - co
                        cos.append(co)
                        ws.append(w)
                        s_t = ps.tile([128, QG], F32, tag="ps", name="ps")
                        s_ts.append(s_t)
                        nc.tensor.matmul(
                            s_t[:, 0:w],
                            kt_sb[b][hs, ck * 128:(ck + 1) * 128],
                            qt_sb[b][hs, qg * QG + co:(qg + 1) * QG],
                            start=True, stop=True)
                    # exp'd scores land in one contiguous [128, T-off] fp16
                    # tile whose column j maps to q = off + j (same mapping
                    # as rp_t), enabling a single fused 4x-mode DVE multiply
                    e_t = epool.tile([128, T], FP16, tag="ee", name="ee")
                    for i, qg in enumerate(range(qg0, NQG)):
                        ej = qg * QG + cos[i] - off
                        nc.scalar.activation(
                            e_t[:, ej:ej + ws[i]], s_ts[i][:, 0:ws[i]],
                            mybir.ActivationFunctionType.Exp,
                            bias=kpad[:, b, ck:ck + 1])
                    p_t = ppool.tile([128, T], FP16, tag="pp", name="pp")
                    nc.vector.tensor_mul(p_t[:, co0:co0 + wt],
                                         e_t[:, co0:co0 + wt],
                                         rp_t[:, co0:co0 + wt])
                    for i, qg in enumerate(range(qg0, NQG)):
                        pj = qg * QG + cos[i] - off
                        nc.tensor.matmul(
                            o_ps[qg][:, cos[i]:QG],
                            vaug[b][:, h, ck * 80:ck * 80 + DK + 1],
                            p_t[:, pj:pj + ws[i]],
                            start=(ck == 0), stop=(ck == 4 * qg + 3))
                    if ck % 4 == 3:
                        norm_dve(u, ck // 4)
                pending.append((u, NQG - 1))
                if u == (0, HPC - 1):
                    # batch 0 fully attended: queue its out-projection
                    # (qg-major so the still-pending qg3 pieces come last;
                    # the pending norm_pe pops at the next unit's ck=4,
                    # before any qg3 piece is emitted)
                    oproj_q.extend((0, db, qg)
                                   for qg in range(NQG) for db in range(NDC))
            while pending:
                norm_pe(*pending.pop(0))
            oproj_q.extend((1, db, NQG - 1) for db in range(NDC))
            while oproj_q:
                oproj_piece(*oproj_q.pop(0))

    nc.compile()
    return nc


def _prep_host(q, k, v, key_pad_mask, attn_mask, relpos_bias, Wq, Wk, Wv, Wo):
    f32, f16 = np.float32, np.float16
    qT = np.asarray(q, f32).transpose(0, 2, 1).astype(f16)
    kT = np.asarray(k, f32).transpose(0, 2, 1).astype(f16)
    vT = np.asarray(v, f32).transpose(0, 2, 1).astype(f16)

    kb = np.where(np.asarray(key_pad_mask), NEG, f32(0)).astype(f32)  # [B,T]
    kpadT = np.ascontiguousarray(kb.reshape(B, NKC, 128).transpose(2, 0, 1))

    maskT = np.asarray(attn_mask).T  # [k, q], True = masked (k > q)
    rp = np.asarray(relpos_bias, f32)

    Wq = np.asarray(Wq, f32) * f32(1.0 / np.sqrt(DK))
    Wk = np.asarray(Wk, f32)
    Wv = np.asarray(Wv, f32)
    Wo = np.asarray(Wo, f32)

    def wlayout(Wrows):  # [128, D] -> contiguous [p, a*128+m] = W.T[a*128+p, m]
        return np.ascontiguousarray(
            Wrows.T.reshape(NDC, 128, 128).transpose(1, 0, 2).reshape(128, D)
        ).astype(f16)

    in_maps = []
    for c in range(NCORES):
        rows = slice(c * 128, (c + 1) * 128)
        h0 = 2 * c
        # exp(relpos^T), causal-masked entries exactly 0, fp16
        exprpT = np.where(maskT[None], f32(0),
                          np.exp(rp[h0:h0 + 2].transpose(0, 2, 1))).astype(f16)
        in_maps.append({
            "qT": qT, "kT": kT, "vT": vT,
            "exprpT": np.ascontiguousarray(exprpT),
            "kpadT": kpadT,
            "wqT": wlayout(Wq[rows]),
            "wkT": wlayout(Wk[rows]),
            "wvT": wlayout(Wv[rows]),
            "woT": np.ascontiguousarray(Wo[:, rows].T).astype(f16),
        })
    return in_maps


def run(trace=False, tmpdir=None, **inputs):
    if "nc" not in _CACHE:
        _CACHE["nc"] = _build_program()
    nc = _CACHE["nc"]
    in_maps = _prep_host(**inputs)
    res = run_bass_kernel_spmd(nc, in_maps, core_ids=list(range(NCORES)),
                               trace=trace, tmpdir=tmpdir)
    acc = res.results[0]["outT"].astype(np.float64)
    for c in range(1, NCORES):
        acc += res.results[c]["outT"]
    out = np.ascontiguousarray(acc.transpose(0, 2, 1)).astype(np.float32)
    return out, res


def kernel(**inputs) -> np.ndarray:
    out, _ = run(trace=False, **inputs)
    return out


# revision 6
# speedup vs baseline: 1.2000x; 1.2000x over previous
"""Multi-head attention (B=2, T=2048, D=1024, H=16) on 8 Trainium2 NeuronCores.

Sharding: tensor-parallel over heads — core c owns global heads {2c, 2c+1} for
both batch elements (Wq/Wk/Wv column-split, Wo row-split, relpos_bias split
along H).  Each core computes a partial [B, D, T] output-projection product;
the host sums the 8 partials and transposes back to [B, T, D].  SPMD: one
program, per-core weight/relpos slices in the input maps; no collectives.

Device-side layout ("transposed flash attention"): scores are computed as
S^T[k, q] so the exp'd scores are already in the right layout (k on
partitions) to be the moving operand of the P@V matmul — the attention
matrix is never transposed on device.

Design notes:
  - fp16 matmuls everywhere; relpos bias is added into the scores PSUM
    accumulation by an identity-stationary fp8 matmul (keeps the PE stream
    dense — a DVE add/mul hop lengthens the critical path and lets the HAM
    duty-cycle governor drop the PE to half rate).  The causal mask is
    baked into relposT on the host as -240.  Fully-masked k-blocks are
    skipped and diagonal-band blocks are column-restricted to the causal
    wavefront.
  - key-pad mask rides the ACT exp instruction as a per-partition bias.
  - softmax max-subtraction is skipped (scores ~N(0,1), exp is safe); the
    denominator comes free as an extra row of the P@V matmul from an
    all-ones column appended to V; 1/sqrt(dk) is folded into Wq.
  - weights are pre-transposed on host into contiguous per-partition
    layouts (2KB rows; the first 128-col chunk of Wq is a separate DMA) so
    the first projection matmul can issue ~10us earlier than a
    256B-descriptor gather allows.
  - the two heads' output projections are merged: at_sb is one [128, T]
    tile per batch (h0 rows 0:64, h1 rows 64:128) and Wo is one [128, D]
    stationary, halving oproj matmul count.
  - normalization: full-tile copy + reciprocal_approx_fast (SBUF source
    only, partition base 0 — single-partition slices at base 64 return
    zeros) + single fp16 cast (5e-4 rel err, inside the 2e-2 budget; no
    hi/lo split) + one ones-outer-product broadcast matmul.
  - output casts alternate DVE / Scalar-ACT-Copy (same act table as Exp,
    no table reload); output DMAs alternate Sync / GpSimd rings.
  - the whole program is one software-pipelined stream: batch-1
    projections, per-q-group normalizations, and output-projection pieces
    are interleaved into the attention k-loops so the PE never idles long
    enough to re-throttle (HAM).
"""

import sys

for p in ("/opt/trn_rl_repo", "/root/.axon_site/_ro/trn_rl_repo"):
    if p not in sys.path:
        sys.path.insert(0, p)

import numpy as np
import ml_dtypes

import concourse.bacc as bacc
import concourse.mybir as mybir
import concourse.tile as tile
from concourse.bass_utils import run_bass_kernel_spmd

B, T, D, H = 2, 2048, 1024, 16
DK = D // H          # 64
NCORES = 8
HPC = H // NCORES    # heads per core = 2
QG = 512             # q-group width
NQG = T // QG        # 4
NKC = T // 128       # 16 k-chunks
NDC = D // 128       # 8 d-chunks
NEG = np.float32(-1e30)

F32 = mybir.dt.float32
FP16 = mybir.dt.float16
FP8 = mybir.dt.float8e4

_CACHE = {}


def _build_program():
    nc = bacc.Bacc("TRN2", target_bir_lowering=False, debug=False,
                   enable_asserts=True)

    d_qT = nc.dram_tensor("qT", [B, D, T], FP16, kind="ExternalInput").ap()
    d_kT = nc.dram_tensor("kT", [B, D, T], FP16, kind="ExternalInput").ap()
    d_vT = nc.dram_tensor("vT", [B, D, T], FP16, kind="ExternalInput").ap()
    d_rp = nc.dram_tensor("relposT", [HPC, T, T], FP8, kind="ExternalInput").ap()
    d_kp = nc.dram_tensor("kpadT", [128, B, NKC], F32, kind="ExternalInput").ap()
    d_wq = nc.dram_tensor("wqT", [128, D], FP16, kind="ExternalInput").ap()
    d_wk = nc.dram_tensor("wkT", [128, D], FP16, kind="ExternalInput").ap()
    d_wv = nc.dram_tensor("wvT", [128, D], FP16, kind="ExternalInput").ap()
    d_wo = nc.dram_tensor("woT", [128, D], FP16, kind="ExternalInput").ap()
    d_id8 = nc.dram_tensor("id8", [128, 128], FP8, kind="ExternalInput").ap()
    d_out = nc.dram_tensor("outT", [B, D, T], FP16, kind="ExternalOutput").ap()

    with tile.TileContext(nc) as tc:
        with (
            tc.tile_pool(name="persist", bufs=1) as persist,
            tc.tile_pool(name="stream", bufs=6) as stream,
            tc.tile_pool(name="rp", bufs=6) as rppool,
            tc.tile_pool(name="ee", bufs=5) as epool,
            tc.tile_pool(name="oc", bufs=6) as ocpool,
            tc.tile_pool(name="nrm", bufs=2) as nrm,
            tc.tile_pool(name="ps", bufs=4, space="PSUM") as ps,
            tc.tile_pool(name="opsum", bufs=4, space="PSUM") as ops,
        ):
            # ---- weights + inputs, first-needed-first, split across rings.
            # wq's first 128-col chunk is its own DMA so the very first
            # matmul (needs only chunk 0 + xq0[0]) can go ASAP; xq0[0]
            # rides the sync ring in parallel with wq on scalar.
            wq = persist.tile([128, NDC, 128], FP16, tag="wq", name="wq")
            d_wq_c = d_wq.rearrange("p (a m) -> p a m", m=128)
            nc.scalar.dma_start(out=wq[:, 0, :], in_=d_wq_c[:, 0, :])

            def load_x(dten, b, dk, eng, tag="xin", bufs=10):
                t = stream.tile([128, T], FP16, tag=tag, bufs=bufs,
                                name=f"x{tag}{b}{dk}")
                eng.dma_start(out=t[:],
                              in_=dten[b, dk * 128:(dk + 1) * 128, :])
                return t

            engs = (nc.sync, nc.scalar)
            xq0 = {dk: load_x(d_qT, 0, dk, engs[dk % 2]) for dk in range(NDC)}
            nc.scalar.dma_start(out=wq[:, 1:NDC, :], in_=d_wq_c[:, 1:NDC, :])

            wv = persist.tile([128, NDC, 128], FP16, tag="wv", name="wv")
            nc.scalar.dma_start(out=wv[:], in_=d_wv.rearrange(
                "p (a m) -> p a m", m=128))
            vch0 = {dk: load_x(d_vT, 0, dk, engs[dk % 2], "xinv", 8)
                    for dk in range(NDC)}

            wk = persist.tile([128, NDC, 128], FP16, tag="wk", name="wk")
            nc.sync.dma_start(out=wk[:], in_=d_wk.rearrange(
                "p (a m) -> p a m", m=128))
            xk0 = {dk: load_x(d_kT, 0, dk, engs[dk % 2]) for dk in range(NDC)}

            w_sb = {"q": wq, "k": wk, "v": wv}

            id8 = persist.tile([128, 128], FP8, tag="id8", name="id8")
            nc.sync.dma_start(out=id8[:], in_=d_id8[:])
            kpad = persist.tile([128, B, NKC], F32, tag="kpad", name="kpad")
            nc.sync.dma_start(out=kpad[:], in_=d_kp[:])
            wo_sb = persist.tile([128, D], FP16, tag="wo", name="wo")
            nc.sync.dma_start(out=wo_sb[:], in_=d_wo[:])

            ones = persist.tile([128, DK], F32, tag="ones", name="ones")
            nc.vector.memset(ones[:], 1.0)
            ones16 = persist.tile([128, DK], FP16, tag="ones16", name="ones16")
            nc.vector.memset(ones16[:], 1.0)

            qt_sb, kt_sb, vaug = {}, {}, {}
            for b in range(B):
                qt_sb[b] = persist.tile([128, T], FP16, tag=f"qt{b}",
                                        name=f"qt{b}")
                kt_sb[b] = persist.tile([128, T], FP16, tag=f"kt{b}",
                                        name=f"kt{b}")
                va = persist.tile([128, HPC, NKC * 80], FP16, tag=f"va{b}",
                                  name=f"va{b}")
                va_c = va[:].rearrange("p h (c u) -> p h c u", u=80)
                for h in range(HPC):
                    nc.vector.tensor_copy(va_c[:, h, :, 64], ones[:, 0:NKC])
                vaug[b] = va

            # ---- projection helpers (weight-stationary, dk outer) ----
            def proj_qk(nm, b, xts, dst):
                accs = [ps.tile([128, QG], F32, tag="ps", name="ps")
                        for _ in range(NQG)]
                for dk in range(NDC):
                    for cc in range(NQG):
                        nc.tensor.matmul(
                            accs[cc][:], w_sb[nm][:, dk, :],
                            xts[dk][:, cc * QG:(cc + 1) * QG],
                            start=(dk == 0), stop=(dk == NDC - 1))
                for cc in range(NQG):
                    if b == 0:
                        # scalar is exp-free in phase 1; DVE keeps batch 1
                        nc.scalar.activation(
                            dst[b][:, cc * QG:(cc + 1) * QG], accs[cc][:],
                            mybir.ActivationFunctionType.Copy)
                    else:
                        nc.vector.tensor_copy(
                            dst[b][:, cc * QG:(cc + 1) * QG], accs[cc][:])

            def proj_v(b, vts):
                for tb in range(NKC):
                    ts_ = slice(tb * 128, (tb + 1) * 128)
                    acc = ps.tile([128, 128], F32, tag="ps", name="psv")
                    for dk in range(NDC):
                        nc.tensor.matmul(
                            acc[:], vts[dk][:, ts_], w_sb["v"][:, dk, :],
                            start=(dk == 0), stop=(dk == NDC - 1))
                    # both heads' 64-wide slices in one strided copy
                    va_c = vaug[b][:].rearrange("p h (c u) -> p h c u", u=80)
                    nc.vector.tensor_copy(
                        va_c[:, :, tb, 0:DK],
                        acc[:].rearrange("p (h u) -> p h u", h=HPC))

            # ---- phase 1: batch 0 projections ----
            proj_qk("q", 0, xq0, qt_sb)
            proj_v(0, vch0)
            proj_qk("k", 0, xk0, kt_sb)

            # ---- phase 2: attention; batch-1 projections and all output
            # projections are interleaved into the instruction stream ----
            at_sb = {}
            for b in range(B):
                at_sb[b] = persist.tile([128, T], FP16, tag=f"at{b}",
                                        name=f"at{b}")

            def norm_dve(u, qg):
                o = nrm.tile([DK + 1, QG], F32, tag="oc2", name="oc2")
                nc.vector.tensor_copy(o[:], pend_ops[u][qg][:])
                rc = nrm.tile([DK + 1, QG], F32, tag="rc", name="rc")
                nc.vector.reciprocal_approx_fast(out=rc[:], in_=o[:])
                rch = nrm.tile([DK + 1, QG], FP16, tag="rch", name="rch")
                nc.vector.tensor_copy(rch[:], rc[:])
                pend_dve[(u, qg)] = (o, rch)

            def norm_pe(u, qg):
                b, h = u
                o, rch = pend_dve.pop((u, qg))
                rb = ops.tile([DK, QG], F32, tag="ops", name="rb")
                nc.tensor.matmul(
                    rb[:], ones16[DK:DK + 1, :], rch[DK:DK + 1, :],
                    start=True, stop=True)
                nc.vector.tensor_mul(
                    at_sb[b][h * DK:(h + 1) * DK, qg * QG:(qg + 1) * QG],
                    o[0:DK, :], rb[:])

            oproj_n = [0]

            def oproj_piece(b, db, qg):
                ds_ = slice(db * 128, (db + 1) * 128)
                qs = slice(qg * QG, (qg + 1) * QG)
                pp = ps.tile([128, QG], F32, tag="ps", name="pp")
                nc.tensor.matmul(pp[:], wo_sb[:, ds_], at_sb[b][:, qs],
                                 start=True, stop=True)
                oc = ocpool.tile([128, QG], FP16, tag="oc", name="oc")
                n = oproj_n[0]
                oproj_n[0] += 1
                # PSUM->SBUF cast alternates DVE / Scalar Copy (same act
                # table as Exp); output DMA alternates Sync / GpSimd rings
                if n % 2 == 0:
                    nc.vector.tensor_copy(oc[:], pp[:])
                else:
                    nc.scalar.activation(oc[:], pp[:],
                                         mybir.ActivationFunctionType.Copy)
                (nc.sync if n % 2 == 0 else nc.gpsimd).dma_start(
                    out=d_out[b, ds_, qs], in_=oc[:])

            pend_ops, pend_dve = {}, {}
            pending = []          # (unit, qg) whose norm_pe is still owed
            oproj_q = []          # (b, db, qg) out-proj pieces ready to emit
            units = [(b, h) for b in range(B) for h in range(HPC)]
            xq1 = vch1 = None
            for ui, u in enumerate(units):
                b, h = u
                hs = slice(h * DK, (h + 1) * DK)
                if ui == 0:
                    # prefetch batch-1 q and v rows.  These issues can wait
                    # a long time (in engine terms) on pool-slot semaphores,
                    # so they MUST NOT sit in the Scalar queue where they
                    # would stall the exp instructions behind them; the
                    # otherwise-mostly-idle GpSimd (SWDGE) queue absorbs
                    # the waits (its later work comes after the slots free).
                    xq1 = {dk: load_x(d_qT, 1, dk, nc.gpsimd)
                           for dk in range(NDC)}
                    vch1 = {dk: load_x(d_vT, 1, dk, nc.gpsimd, "xinv", 8)
                            for dk in range(NDC)}
                if ui == 1:
                    # batch-1 projections: data already resident
                    proj_qk("q", 1, xq1, qt_sb)
                    xk1 = {dk: load_x(d_kT, 1, dk, nc.gpsimd)
                           for dk in range(NDC)}
                    proj_v(1, vch1)
                    proj_qk("k", 1, xk1, kt_sb)
                pend_ops[u] = [ops.tile([DK + 1, QG], F32, tag="ops",
                                        name="ops") for _ in range(NQG)]
                o_ps = pend_ops[u]
                for ck in range(NKC):
                    if ck == 4 and pending:
                        pu, pqg = pending.pop(0)
                        norm_pe(pu, pqg)
                        if pu[1] == HPC - 1:
                            # previous batch's qg3 pieces now unblocked
                            oproj_q.extend((pu[0], db, pqg)
                                           for db in range(NDC))
                    if ck >= 5 and (ck - 5) % 4 == 0:
                        qgn = (ck - 5) // 4
                        norm_pe(u, qgn)
                        if h == HPC - 1 and qgn < NQG - 1:
                            # both heads of batch b normalized for qgn:
                            # its out-projection pieces are ready
                            oproj_q.extend((b, db, qgn) for db in range(NDC))
                    if ck >= 4 and oproj_q:
                        oproj_piece(*oproj_q.pop(0))
                        if oproj_q and (len(oproj_q) > 12 or ck % 2 == 0):
                            oproj_piece(*oproj_q.pop(0))
                    qg0 = ck // 4          # first valid q-group
                    off = qg0 * QG         # start col of rp tile
                    rp_t = rppool.tile([128, T], FP8, tag="rp", name="rp")
                    nc.sync.dma_start(
                        out=rp_t[:, 0:T - off],
                        in_=d_rp[h, ck * 128:(ck + 1) * 128, off:T])
                    s_ts, cos, ws = [], [], []
                    for qg in range(qg0, NQG):
                        co = max(0, ck * 128 - qg * QG)
                        w = QG - co
                        cos.append(co)
                        ws.append(w)
                        s_t = ps.tile([128, QG], F32, tag="ps", name="ps")
                        s_ts.append(s_t)
                        nc.tensor.matmul(
                            s_t[:, 0:w],
                            kt_sb[b][hs, ck * 128:(ck + 1) * 128],
                            qt_sb[b][hs, qg * QG + co:(qg + 1) * QG],
                            start=True, stop=False)
                    for i, qg in enumerate(range(qg0, NQG)):
                        rj = qg * QG + cos[i] - off
                        nc.tensor.matmul(
                            s_ts[i][:, 0:ws[i]], id8[:],
                            rp_t[:, rj:rj + ws[i]],
                            start=False, stop=True)
                    e_ts = []
                    for i, qg in enumerate(range(qg0, NQG)):
                        e_t = epool.tile([128, QG], FP16, tag="ee", name="ee")
                        e_ts.append(e_t)
                        nc.scalar.activation(
                            e_t[:, 0:ws[i]], s_ts[i][:, 0:ws[i]],
                            mybir.ActivationFunctionType.Exp,
                            bias=kpad[:, b, ck:ck + 1])
                    for i, qg in enumerate(range(qg0, NQG)):
                        nc.tensor.matmul(
                            o_ps[qg][:, cos[i]:QG],
                            vaug[b][:, h, ck * 80:ck * 80 + DK + 1],
                            e_ts[i][:, 0:ws[i]],
                            start=(ck == 0), stop=(ck == 4 * qg + 3))
                    if ck % 4 == 3:
                        norm_dve(u, ck // 4)
                pending.append((u, NQG - 1))
            while pending:
                pu, pqg = pending.pop(0)
                norm_pe(pu, pqg)
                if pu[1] == HPC - 1:
                    oproj_q.extend((pu[0], db, pqg) for db in range(NDC))
            while oproj_q:
                oproj_piece(*oproj_q.pop(0))

    nc.compile()
    return nc


def _prep_host(q, k, v, key_pad_mask, attn_mask, relpos_bias, Wq, Wk, Wv, Wo):
    f32, f16 = np.float32, np.float16
    qT = np.asarray(q, f32).transpose(0, 2, 1).astype(f16)
    kT = np.asarray(k, f32).transpose(0, 2, 1).astype(f16)
    vT = np.asarray(v, f32).transpose(0, 2, 1).astype(f16)

    kb = np.where(np.asarray(key_pad_mask), NEG, f32(0)).astype(f32)  # [B,T]
    kpadT = np.ascontiguousarray(kb.reshape(B, NKC, 128).transpose(2, 0, 1))

    maskT = np.asarray(attn_mask).T  # [k, q], True = masked (k > q)
    rp = np.asarray(relpos_bias, f32)

    id8 = np.eye(128, dtype=ml_dtypes.float8_e4m3)

    Wq = np.asarray(Wq, f32) * f32(1.0 / np.sqrt(DK))
    Wk = np.asarray(Wk, f32)
    Wv = np.asarray(Wv, f32)
    Wo = np.asarray(Wo, f32)

    def wlayout(Wrows):  # [128, D] -> contiguous [p, a*128+m] = W.T[a*128+p, m]
        return np.ascontiguousarray(
            Wrows.T.reshape(NDC, 128, 128).transpose(1, 0, 2).reshape(128, D)
        ).astype(f16)

    in_maps = []
    for c in range(NCORES):
        rows = slice(c * 128, (c + 1) * 128)
        h0 = 2 * c
        rpT = np.where(maskT[None], f32(-240.0),
                       rp[h0:h0 + 2].transpose(0, 2, 1)).astype(
                           ml_dtypes.float8_e4m3)
        in_maps.append({
            "qT": qT, "kT": kT, "vT": vT,
            "relposT": np.ascontiguousarray(rpT),
            "kpadT": kpadT,
            "wqT": wlayout(Wq[rows]),
            "wkT": wlayout(Wk[rows]),
            "wvT": wlayout(Wv[rows]),
            "woT": np.ascontiguousarray(Wo[:, rows].T).astype(f16),
            "id8": id8,
        })
    return in_maps


def run(trace=False, tmpdir=None, **inputs):
    if "nc" not in _CACHE:
        _CACHE["nc"] = _build_program()
    nc = _CACHE["nc"]
    in_maps = _prep_host(**inputs)
    res = run_bass_kernel_spmd(nc, in_maps, core_ids=list(range(NCORES)),
                               trace=trace, tmpdir=tmpdir)
    acc = res.results[0]["outT"].astype(np.float64)
    for c in range(1, NCORES):
        acc += res.results[c]["outT"]
    out = np.ascontiguousarray(acc.transpose(0, 2, 1)).astype(np.float32)
    return out, res


def kernel(**inputs) -> np.ndarray:
    out, _ = run(trace=False, **inputs)
    return out
